# revision 1
# baseline (speedup 1.0000x reference)
"""Trainium2 Bass kernel for nn_Attention_85074712199827.

Computes, for hidden [1,32,1024], encoder_outputs [32,2048,1024],
W_attn [1024,2048], b_attn [1024], v [1024]:

    h_proj  = hidden[0] @ W_attn[:, :1024].T
    e_proj  = encoder_outputs @ W_attn[:, 1024:].T
    energy  = tanh(e_proj + h_proj[:, None, :] + b_attn)
    att     = energy @ v
    out     = softmax(att, axis=1)          # [32, 2048] float32

Distribution: data-parallel over the batch across 8 NeuronCores (4
batch rows per core); the tiny parameters are replicated (pre-laid-out
and pre-cast to bf16 on the host). Each core runs an independent
Bass/Tile program; results are concatenated on the host.

Per-core schedule: enc streams HBM->SBUF fp32 (gpsimd queue), is cast
to bf16 (DVE), xbar-transposed to [h, s] layout (sync queue), then the
PE runs the e_proj matmuls with 1024-wide moving operands.  The v-dot
is NOT done with 1-row PE matmuls: the tanh output is scaled by v on
the scalar engine (Copy activation with per-partition scale), the 8
o-chunks are summed on DVE, and a single ones-vector matmul reduces
the 128 partitions.  Softmax runs per batch row as soon as that row's
attention logits are complete, overlapped with the next row's matmuls.

Self-contained: only environment packages (concourse, numpy, ml_dtypes)
are imported; all shapes/sharding are hardcoded for this problem.
"""

from contextlib import ExitStack

import ml_dtypes
import numpy as np

import concourse.bass as bass
import concourse.tile as tile
from concourse import bacc, mybir

F32 = mybir.dt.float32
BF16 = mybir.dt.bfloat16
AF = mybir.ActivationFunctionType
ADD = mybir.AluOpType.add
P = 128


def build_nc(b_loc=4, s=2048, h=1024, n_cores=8, sb=512,
             warmup_mm=40, startup_keep=12, keepalive_mm=0,
             first_sb=512, ld_chunk=512):
    n_hc = h // P           # contraction chunks
    n_ot = h // P           # output (o) tiles
    SC = sb                 # matmul moving width == s-block

    nc = bacc.Bacc("TRN2", target_bir_lowering=False, debug=False,
                   num_devices=n_cores)

    wt = nc.dram_tensor("wt", [2 * h, h], BF16, kind="ExternalInput").ap()
    hiddenT = nc.dram_tensor("hiddenT", [h, b_loc], BF16, kind="ExternalInput").ap()
    b_attn = nc.dram_tensor("b_attn", [h], F32, kind="ExternalInput").ap()
    v = nc.dram_tensor("v", [h], F32, kind="ExternalInput").ap()
    ones = nc.dram_tensor("ones", [P, 1], BF16, kind="ExternalInput").ap()
    zeros = nc.dram_tensor("zeros", [P, 512], BF16, kind="ExternalInput").ap()
    enc = nc.dram_tensor("enc", [b_loc, s, h], F32, kind="ExternalInput").ap()
    out = nc.dram_tensor("out", [b_loc, s], F32, kind="ExternalOutput").ap()

    with tile.TileContext(nc) as tc, ExitStack() as ctx:
        const = ctx.enter_context(tc.tile_pool(name="const", bufs=1))
        psmall = ctx.enter_context(tc.tile_pool(name="psmall", bufs=1, space="PSUM"))

        # ---- PE warmup: dependency-free matmuls to lift the HAM clock
        # gate to 8/8 while the first enc block is still in flight ----
        wz = const.tile([P, 512], BF16)
        nc.scalar.dma_start(wz[:], zeros)
        for i in range(warmup_mm):
            pw = psmall.tile([P, 512], F32, name="pw", tag="ps")
            nc.tensor.matmul(pw[:], wz[:, :P], wz[:], start=True, stop=True)

        def keepalive(n):
            for _ in range(n):
                pw = psmall.tile([P, 512], F32, name="pw", tag="ps")
                nc.tensor.matmul(pw[:], wz[:, :P], wz[:], start=True, stop=True)

        # ---- small constants (scalar queue; off the critical path) ----
        hT_bf = const.tile([P, n_hc, b_loc], BF16)
        nc.scalar.dma_start(hT_bf[:], hiddenT.rearrange("(hc p) b -> p hc b", p=P))

        baT = const.tile([P, n_ot], F32)
        nc.scalar.dma_start(baT[:], b_attn.rearrange("(oc p) -> p oc", p=P))

        vt_f = const.tile([P, n_ot], F32)
        nc.scalar.dma_start(vt_f[:], v.rearrange("(oc p) -> p oc", p=P))

        ones_bf = const.tile([P, 1], BF16)
        nc.scalar.dma_start(ones_bf[:], ones)

        # ---- weights: W_attn.T arrives [2h, h] bf16; We half first so
        # e_proj unblocks while Wh still streams ----
        wt_bf = const.tile([P, 2 * n_hc, h], BF16)
        wt_r = wt.rearrange("(jc p) o -> p jc o", p=P)
        q = n_hc // 2

        def emit_w(c):
            nc.scalar.dma_start(
                wt_bf[:, c * q:(c + 1) * q, :],
                wt_r[:, c * q:(c + 1) * q, :])

        emit_w(2)
        emit_w(3)

        def emit_hproj():
            hb = const.tile([P, n_ot, b_loc], F32, name="hb")
            for ot in range(n_ot):
                ph = psmall.tile([P, b_loc], F32, name="ph", tag="ps")
                for hc in range(n_hc):
                    nc.tensor.matmul(
                        ph[:], wt_bf[:, hc, ot * P:(ot + 1) * P], hT_bf[:, hc, :],
                        start=(hc == 0), stop=(hc == n_hc - 1))
                nc.vector.tensor_tensor(
                    hb[:, ot, :], ph[:],
                    baT[:, ot, None].to_broadcast((P, b_loc)),
                    mybir.AluOpType.add)
            return hb

        # ---- main pipeline pools ----
        inp = ctx.enter_context(tc.tile_pool(name="inp", bufs=3))
        bfp = ctx.enter_context(tc.tile_pool(name="bfp", bufs=3))
        encT_p = ctx.enter_context(tc.tile_pool(name="encT", bufs=3))
        en_p = ctx.enter_context(tc.tile_pool(name="energy", bufs=4))
        tmp_p = ctx.enter_context(tc.tile_pool(name="vtmp", bufs=4))
        acc_p = ctx.enter_context(tc.tile_pool(name="acc", bufs=3))
        row_p = ctx.enter_context(tc.tile_pool(name="rowbuf", bufs=2))
        pe_p = ctx.enter_context(tc.tile_pool(name="psum_e", bufs=2, space="PSUM"))
        pa_p = ctx.enter_context(tc.tile_pool(name="psum_att", bufs=2, space="PSUM"))

        att_rows = [const.tile([1, s], F32, name=f"attrow{b}")
                    for b in range(b_loc)]

        # units: (b, s0, size) — b-major so softmax(b) pipelines.
        # The first units of b=0 are small for a fast pipeline rampup.
        units = []
        for b in range(b_loc):
            if b == 0 and first_sb < sb:
                for s0 in range(0, s, first_sb):
                    units.append((b, s0, first_sb))
            else:
                for s0 in range(0, s, sb):
                    units.append((b, s0, sb))

        def phase1(unit):
            # HBM -> SBUF fp32 loads on the sync HWDGE queue (SWDGE issue
            # on gpsimd measured 10-40us/load), then DVE casts to bf16.
            b, s0, sz = unit
            its = []
            for c0 in range(0, sz, ld_chunk):
                csz = min(ld_chunk, sz - c0)
                it = inp.tile([P, csz // P, h], F32, name="it")
                nc.sync.dma_start(
                    it[:], enc[b, s0 + c0:s0 + c0 + csz, :].rearrange(
                        "(si p) h -> p si h", p=P))
                its.append(it)
            bts = []
            for it in its:
                bt = bfp.tile([P, it.shape[1], h], BF16, name="bt")
                nc.vector.tensor_copy(out=bt[:], in_=it[:])
                bts.append(bt)
            return bts

        def phase2(unit, bts):
            # SBUF xbar transpose [s,h] bf16 -> [h,s], alternating between
            # the sync and scalar HWDGE queues to halve the serial latency
            b, s0, sz = unit
            eT = encT_p.tile([P, n_hc, sz], BF16, name="eT")
            col = 0
            for bt in bts:
                for si in range(bt.shape[1]):
                    nc.sync.dma_start_transpose(
                        eT[:, :, col:col + P], bt[:, si, :])
                    col += P
            return eT

        def phase3_mm(unit, eT, hb):
            # PSUM matmul output must fit one 2KB bank -> 512-wide chunks.
            # tanh on ACT, x v_o and the ot-accumulation on DVE.
            b, s0, sz = unit
            acc = acc_p.tile([P, sz], BF16, name="acc")
            for ot in range(n_ot):
                eng = en_p.tile([P, sz], BF16, name="eng")
                pe = pe_p.tile([P, sz], F32, name="pe")
                for c0 in range(0, sz, 512):
                    for hc in range(n_hc):
                        nc.tensor.matmul(
                            pe[:, c0:c0 + 512],
                            wt_bf[:, n_hc + hc, ot * P:(ot + 1) * P],
                            eT[:, hc, c0:c0 + 512],
                            start=(hc == 0), stop=(hc == n_hc - 1))
                nc.scalar.activation(
                    eng[:], pe[:], AF.Tanh, bias=hb[:, ot, b:b + 1])
                if ot == 0:
                    nc.vector.tensor_scalar_mul(acc[:], eng[:], vt_f[:, 0:1])
                else:
                    tmp = tmp_p.tile([P, sz], BF16, name="tmp")
                    nc.vector.tensor_scalar_mul(
                        tmp[:], eng[:], vt_f[:, ot:ot + 1])
                    nc.vector.tensor_tensor(acc[:], acc[:], tmp[:], ADD)
            return acc

        def phase3_fin(unit, acc):
            # partition-reduce via ones-matmul (emitted one unit late so
            # the PE never waits on the DVE acc), then DVE copies the
            # logits PSUM->SBUF.  This copy is the only op that waits on
            # the late ones-matmul, and it sits LAST in the DVE FIFO for
            # this iteration, so nothing upstream ever blocks behind it.
            b, s0, sz = unit
            for c0 in range(0, sz, 512):
                pa = pa_p.tile([P, 512], F32, name="pa")
                nc.tensor.matmul(
                    pa[0:1, :], ones_bf[:, 0:1], acc[:, c0:c0 + 512],
                    start=True, stop=True)
                nc.vector.tensor_copy(
                    out=att_rows[b][0:1, s0 + c0:s0 + c0 + 512],
                    in_=pa[0:1, :])

        def softmax_row(b):
            # Runs 2+ units after row b's logits landed in SBUF, so every
            # input is long ready when each queue reaches these ops.
            # |att| < ~6, so exp() is safe in fp32 without the row max.
            e_row = row_p.tile([1, s], F32, name="erow")
            ssum = const.tile([1, 1], F32, name=f"ssum{b}")
            nc.scalar.activation(
                e_row[:], att_rows[b][:], AF.Exp, accum_out=ssum[:])
            rinv = const.tile([1, 1], F32, name=f"rinv{b}")
            nc.vector.reciprocal(rinv[:], ssum[:])
            nc.vector.tensor_scalar_mul(e_row[:], e_row[:], rinv[:])
            nc.gpsimd.dma_start(out[b:b + 1, :], e_row[:])

        # ---- software pipeline, 3-deep load lookahead:
        #   iter i: matmuls(u_i) | transposes(u_{i+1}) | loads+casts(u_{i+3})
        #           | fin(u_{i-1}) | softmax(row done at u_{i-2})
        # Casts sit in the DVE queue AFTER acc-adds(u_i) and BEFORE
        # fin(u_{i-1})'s psum copy, so no producer ever waits behind a
        # consumer of the late ones-matmul.
        LOOK = 3
        bts_q = {}
        bts_q[0] = phase1(units[0])
        eT_cur = phase2(units[0], bts_q.pop(0))
        emit_w(0)
        emit_w(1)
        hb = emit_hproj()
        for k in range(1, min(LOOK, len(units))):
            bts_q[k] = phase1(units[k])

        fin = None
        sm_row = None
        for i, u in enumerate(units):
            acc = phase3_mm(u, eT_cur, hb)
            if i + 1 < len(units):
                eT_cur = phase2(units[i + 1], bts_q.pop(i + 1))
                if i + LOOK < len(units):
                    bts_q[i + LOOK] = phase1(units[i + LOOK])
            if sm_row is not None:
                softmax_row(sm_row)
                sm_row = None
            if fin is not None:
                phase3_fin(*fin)
                fb, fs0, fsz = fin[0]
                if fs0 + fsz == s:
                    sm_row = fb
            fin = (u, acc)
            if i == 0:
                keepalive(startup_keep)
            else:
                keepalive(keepalive_mm)
        phase3_fin(*fin)
        softmax_row(b_loc - 1)

    nc.compile()
    return nc


def make_in_maps(hidden, encoder_outputs, W_attn, b_attn, v, n_cores=8):
    hidden = np.asarray(hidden, dtype=np.float32)
    encoder_outputs = np.asarray(encoder_outputs, dtype=np.float32)
    W_attn = np.asarray(W_attn, dtype=np.float32)
    b_attn = np.asarray(b_attn, dtype=np.float32)
    v = np.asarray(v, dtype=np.float32)

    b = encoder_outputs.shape[0]
    b_loc = b // n_cores
    wt = np.ascontiguousarray(W_attn.T.astype(ml_dtypes.bfloat16))
    ones = np.ones((P, 1), dtype=ml_dtypes.bfloat16)
    in_maps = []
    for i in range(n_cores):
        bsl = slice(b_loc * i, b_loc * (i + 1))
        in_maps.append({
            "wt": wt,
            "hiddenT": np.ascontiguousarray(
                hidden[0, bsl].T.astype(ml_dtypes.bfloat16)),
            "b_attn": b_attn,
            "v": v,
            "ones": ones,
            "zeros": np.zeros((P, 512), dtype=ml_dtypes.bfloat16),
            "enc": np.ascontiguousarray(encoder_outputs[bsl]),
        })
    return in_maps


_NC_CACHE = {}


def _get_nc():
    if "nc" not in _NC_CACHE:
        _NC_CACHE["nc"] = build_nc(b_loc=4, s=2048, h=1024, n_cores=8)
    return _NC_CACHE["nc"]


def kernel(hidden, encoder_outputs, W_attn, b_attn, v):
    from concourse.bass_utils import run_bass_kernel_spmd

    nc = _get_nc()
    in_maps = make_in_maps(hidden, encoder_outputs, W_attn, b_attn, v,
                           n_cores=8)
    res = run_bass_kernel_spmd(nc, in_maps, core_ids=list(range(8)))
    out = np.concatenate([np.asarray(res.results[i]["out"])
                          for i in range(8)], axis=0)
    return out.astype(np.float32)

